# revision 27
# baseline (speedup 1.0000x reference)
"""Mixture-of-logistics NLL loss (reduction=mean) on 8 Trainium2 NeuronCores.

Math (per row, K=16 mixture components):
    log_prob = ln(sum_k e^{w_k} pdf_k) - ln(sum_k e^{w_k})
    pdf_k = logistic_pdf(t; loc_k, s_k) = rp_k * sech^2(z_k/2) / 4,
            z_k = (t - loc_k) * rp_k,  rp = 1/s
    sech^2(z/2) = 1 - tanh^2(z/2)
The 1/4 factor is pulled out of the per-row sum and folded into the host
combine as a single -ln(4).

Design (v11; evolved via hardware traces of v1..v10):
Measured engine rates (c=256 chunk, [128,256,16] bf16 tiles):
  DVE tensor_tensor 2x  2.29us | broadcast/mixed-dtype 1x  4.42us
  DVE tensor_scalar 4x  1.22us | tensor_reduce 1x (tree16 is 2x faster)
  DVE reciprocal_approx_fast (custom op, f32 in bf16 out)  ~4.3us
  ACT any activation ~3.7us (1 elem/cycle + ~290ns overhead, dtype-blind)
  ACT table-set switch ~2.7us
Earlier versions balanced 1/s between ACT (ln+exp, table set 6) and DVE
(custom op).  The set6<->set0 rotation required batching groups with a
globally pinned ACT order, and every group seam turned DMA jitter into
a 10-30us cross-engine stall.  v11 drops the ACT path entirely:
 - rp = 1/s via ONE custom-DVE reciprocal_approx_fast per chunk (f32
   scale streamed on the HWDGE sync queue, bf16 out).
 - ACT runs only Exp(w), Copy(t broadcast over K), Tanh(z/2), Square --
   every op in the one `exp_and_others` table set -> zero steady-state
   table loads, no groups, no seams.  The Copy keeps the t-loc subtract
   at 2x on DVE (broadcast APs run at 1x).
 - DVE per chunk: recip, diff = tbc - loc, z = diff*rp, pw = rp*e^w,
   tree16(e^w), then after ACT's Tanh/Square: nsq = 1-th^2
   (tensor_scalar), term = nsq*pw, tree16(term).
 - Anti-ping-pong: ACT chain is pinned one chunk ahead of the DVE tail
   ([ExpW_i, Copy_i, Tanh_{i-1}, Sq_{i-1}]), so neither engine stalls
   on the other's current chunk.
 - w/loc/t stream as f32->bf16 SWDGE cast DMAs on gpsimd (GpSimd does
   only descgen: its tensor ops lock the SBUF port shared with DVE).
 - Validated end-to-end ~3e-4 rel error vs the fp jax reference.

Sharding: pure data parallel over rows (batch*seq) across 8 cores; each core
returns per-partition partial sums [128, 2] = (sum ln(num), sum ln(den));
host combines (mean - ln 4).
"""

import numpy as np

import concourse.bacc as bacc
import concourse.mybir as mybir
import concourse.tile as tile
from concourse.tile_rust import add_dep_helper
from concourse.bass_utils import run_bass_kernel_spmd

B, T, K = 16, 131072, 16
N = B * T                 # 2097152 rows total
NCORES = 8
NLOC = N // NCORES        # 262144 rows per core
P = 128                   # SBUF partitions

F32 = mybir.dt.float32
BF16 = mybir.dt.bfloat16
AF = mybir.ActivationFunctionType
OP = mybir.AluOpType


def build_kernel(nloc=NLOC, chunks=None):
    """Build the per-core Bass module."""
    p = P
    r = nloc // p             # rows per partition (2048)
    if chunks is None:
        chunks = [16, 32, 64, 96, 160, 224, 256, 256, 256, 256, 256, 128, 48]
    assert sum(chunks) == r and nloc % p == 0
    cmax = max(chunks)

    nc = bacc.Bacc("TRN2", target_bir_lowering=False, debug=False)
    w_d = nc.dram_tensor("w", [nloc, K], F32, kind="ExternalInput")
    loc_d = nc.dram_tensor("loc", [nloc, K], F32, kind="ExternalInput")
    scale_d = nc.dram_tensor("scale", [nloc, K], F32, kind="ExternalInput")
    t_d = nc.dram_tensor("t", [nloc], F32, kind="ExternalInput")
    out_d = nc.dram_tensor("out", [p, 2], F32, kind="ExternalOutput")

    wv = w_d.ap().rearrange("(p r) k -> p r k", p=p)
    lv = loc_d.ap().rearrange("(p r) k -> p r k", p=p)
    sv = scale_d.ap().rearrange("(p r) k -> p r k", p=p)
    tv = t_d.ap().rearrange("(p r) -> p r", p=p)

    acts = []  # every ACT instruction, in required execution order

    def act(*args, **kwargs):
        ins = nc.scalar.activation(*args, **kwargs)
        acts.append(ins)
        return ins

    with tile.TileContext(nc) as tc:
        with (
            tc.tile_pool(name="persist", bufs=1) as pp,
            tc.tile_pool(name="prp", bufs=3) as prp,      # rp (bf16)
            tc.tile_pool(name="psc32", bufs=3) as psc32,  # f32 scale
            tc.tile_pool(name="pwld", bufs=4) as pwld,
            tc.tile_pool(name="plc", bufs=4) as plc,
            tc.tile_pool(name="ptb", bufs=3) as ptb,
            tc.tile_pool(name="pt", bufs=2) as pt,
            tc.tile_pool(name="pfill", bufs=2) as pfill,
            nc.allow_low_precision("bf16 pipeline validated: ~3e-4 rel"),
        ):
            t_all = pp.tile([p, r], BF16)         # targets (bf16)
            stash_n = pp.tile([p, r], F32)        # per-row numerator sums
            stash_d = pp.tile([p, r], F32)        # per-row denominator sums
            out_sb = pp.tile([p, 2], F32)

            # Table warm-up: trigger the exp_and_others table load on a
            # tiny dummy tile while the input DMAs stream.
            dummy = pp.tile([p, 8], F32)
            nc.gpsimd.memset(dummy, 1.0)
            act(out=dummy, in_=dummy, func=AF.Tanh)

            def tree16(src, dst_slice, c):
                """Sum src [p, c, 16] bf16 over last axis -> dst_slice [p, c] f32."""
                t1 = pt.tile([p, cmax, 8], BF16, tag="t1", name="t1")[:, :c, :]
                nc.vector.tensor_add(out=t1, in0=src[:, :, 0:8], in1=src[:, :, 8:16])
                t2 = pt.tile([p, cmax, 4], BF16, tag="t2", name="t2")[:, :c, :]
                nc.vector.tensor_add(out=t2, in0=t1[:, :, 0:4], in1=t1[:, :, 4:8])
                t3 = pt.tile([p, cmax, 2], BF16, tag="t3", name="t3")[:, :c, :]
                nc.vector.tensor_add(out=t3, in0=t2[:, :, 0:2], in1=t2[:, :, 2:4])
                nc.vector.tensor_add(out=dst_slice, in0=t3[:, :, 0], in1=t3[:, :, 1])

            from concourse.dve_ops import (
                RECIP_APPROX_FAST_CONSTS,
                RECIPROCAL_APPROX_FAST,
            )
            cns = RECIP_APPROX_FAST_CONSTS

            def emit_pre(sl, c, fill=False):
                """DMAs + ACT lookahead pair + DVE pre-tanh chain.

                fill=True (tiny first chunks): w/loc stream as raw f32 on
                the HWDGE sync queue, which starts moving data several us
                before the gpsimd SWDGE queue comes up, and are cast to
                bf16 by two (tiny) DVE copies.
                """
                sc32 = psc32.tile([p, cmax, K], F32, tag="s32", name="s32t")[:, :c, :]
                rp_t = prp.tile([p, cmax, K], BF16, tag="rp", name="rpt")[:, :c, :]
                w_t = pwld.tile([p, cmax, K], BF16, tag="w", name="wt")[:, :c, :]
                loc_t = plc.tile([p, cmax, K], BF16, tag="loc", name="loct")[:, :c, :]
                tbc = ptb.tile([p, cmax, K], BF16, tag="tb", name="tbt")[:, :c, :]
                nc.sync.dma_start(out=sc32, in_=sv[:, sl, :])    # raw f32, HWDGE
                if fill:
                    wf = pfill.tile([p, 32, K], F32, tag="wf", name="wf")[:, :c, :]
                    lf = pfill.tile([p, 32, K], F32, tag="lf", name="lf")[:, :c, :]
                    nc.sync.dma_start(out=wf, in_=wv[:, sl, :])
                    nc.sync.dma_start(out=lf, in_=lv[:, sl, :])
                    tf = pfill.tile([p, 32], F32, tag="tf", name="tf")[:, :c]
                    nc.sync.dma_start(out=tf, in_=tv[:, sl])
                    nc.vector.tensor_copy(out=w_t, in_=wf)       # f32 -> bf16
                    nc.vector.tensor_copy(out=loc_t, in_=lf)
                    nc.vector.tensor_copy(out=t_all[:, sl], in_=tf)
                else:
                    nc.gpsimd.dma_start(out=w_t, in_=wv[:, sl, :])   # f32->bf16
                    nc.gpsimd.dma_start(out=loc_t, in_=lv[:, sl, :])
                    nc.gpsimd.dma_start(out=t_all[:, sl], in_=tv[:, sl])

                act(out=w_t, in_=w_t, func=AF.Exp)               # e^w
                tb = t_all[:, sl].unsqueeze(2).broadcast_to([p, c, K])
                act(out=tbc, in_=tb, func=AF.Copy)               # t bcast

                nc.vector._custom_dve(
                    RECIPROCAL_APPROX_FAST, out=rp_t, in0=sc32,
                    s0=cns["s0"], s1=cns["s1"], imm2=cns["imm2"],
                )                                                # rp = 1/s
                nc.vector.tensor_sub(out=loc_t, in0=tbc, in1=loc_t)   # diff
                nc.vector.tensor_mul(out=loc_t, in0=loc_t, in1=rp_t)  # z
                tree16(w_t, stash_d[:, sl], c)                   # sum e^w
                nc.vector.tensor_mul(out=rp_t, in0=rp_t, in1=w_t)     # pw
                return loc_t, rp_t

            def emit_tanh(pend):
                sl, c, loc_t, pw_t = pend
                act(out=loc_t, in_=loc_t, func=AF.Tanh, scale=0.5)    # th
                act(out=loc_t, in_=loc_t, func=AF.Square)             # th^2

            def emit_post(pend):
                sl, c, loc_t, pw_t = pend
                nc.vector.tensor_scalar(
                    out=loc_t, in0=loc_t, scalar1=-1.0, scalar2=1.0,
                    op0=OP.mult, op1=OP.add,
                )                                                 # 1 - th^2
                nc.vector.tensor_mul(out=loc_t, in0=loc_t, in1=pw_t)  # term
                tree16(loc_t, stash_n[:, sl], c)

            # Software pipeline: chunk i's Tanh/Square + DVE tail trail
            # chunk i+1's head by one chunk.
            pend = None
            o = 0
            for ci, c in enumerate(chunks):
                sl = slice(o, o + c)
                o += c
                loc_t, pw_t = emit_pre(sl, c, fill=(ci < 2))
                if pend is not None:
                    emit_tanh(pend)
                    emit_post(pend)
                pend = (sl, c, loc_t, pw_t)
            emit_tanh(pend)
            emit_post(pend)

            # ---- per-row logs + per-partition accumulation ----
            act(out=stash_n, in_=stash_n, func=AF.Ln, accum_out=out_sb[:, 0:1])
            act(out=stash_d, in_=stash_d, func=AF.Ln, accum_out=out_sb[:, 1:2])
            nc.gpsimd.dma_start(out=out_d.ap(), in_=out_sb)

            # Pin ACT execution order (same engine -> scheduler-only edges)
            for prev, nxt in zip(acts, acts[1:]):
                add_dep_helper(nxt.ins, prev.ins, False, "act-table-order")

    nc.compile()
    return nc


def _combine(outs, n_rows):
    total = 0.0
    for o in outs:
        total += float(o[:, 0].sum(dtype=np.float64))
        total -= float(o[:, 1].sum(dtype=np.float64))
    return np.float32(total / n_rows - np.log(4.0))


def make_in_maps(weight, loc, scale, targets):
    w = np.ascontiguousarray(weight.reshape(N, K), dtype=np.float32)
    l = np.ascontiguousarray(loc.reshape(N, K), dtype=np.float32)
    s = np.ascontiguousarray(scale.reshape(N, K), dtype=np.float32)
    t = np.ascontiguousarray(targets.reshape(N), dtype=np.float32)
    in_maps = []
    for ci in range(NCORES):
        rs = slice(ci * NLOC, (ci + 1) * NLOC)
        in_maps.append({
            "w": np.ascontiguousarray(w[rs]),
            "loc": np.ascontiguousarray(l[rs]),
            "scale": np.ascontiguousarray(s[rs]),
            "t": np.ascontiguousarray(t[rs]),
        })
    return in_maps


def run(in_maps, **kwargs):
    nc = build_kernel()
    return run_bass_kernel_spmd(nc, in_maps, core_ids=list(range(NCORES)), **kwargs)


def kernel(weight, loc, scale, targets):
    in_maps = make_in_maps(weight, loc, scale, targets)
    last = None
    for _ in range(3):  # rare transient NRT device errors: retry
        try:
            res = run(in_maps)
            return _combine([r["out"] for r in res.results], N)
        except Exception as e:  # noqa: BLE001
            last = e
    raise last


if __name__ == "__main__":
    nc = build_kernel()
    print("kernel built OK")


# revision 28
# speedup vs baseline: 1.0327x; 1.0327x over previous
"""Mixture-of-logistics NLL loss (reduction=mean) on 8 Trainium2 NeuronCores.

Math (per row, K=16 mixture components):
    log_prob = logsumexp_k(logw_k + comp_k) where logw = log_softmax(w)
             = log(sum_k e^{w_k} * pdf_k) - log(sum_k e^{w_k})
    pdf_k = logistic_pdf(t; loc_k, s_k) = (1 - tanh^2(z_k/2)) / (4 s_k),
            z_k = (t - loc_k)/s_k
Using rp = 1/s = exp(-ln(s)):
    pdf = (1 - th^2)/4 * rp,  th = tanh(0.5 * (t - loc) * rp)
    term = e^w * pdf = ((1-th^2)/4) * (rp * e^w)
Output = mean over all rows of log_prob.

Sharding: pure data parallel over rows (batch*seq) across 8 cores; each core
returns per-partition partial sums [128, 2] = (sum ln(num), sum ln(den));
host combines.

ACT table-set discipline (a set switch costs ~1.3us table DMA; walrus maps
ln and exp to different sets, so Lns are batched per chunk):
  phase A (per chunk): Ln(scale) x2, then Exp(-u)/Exp(w)    (2 table loads)
  phase B (per chunk): Tanh (+ Square, which is in every set)
  phase C (end): Ln of row-sums + accumulate
Chunks are software-pipelined one deep (A of chunk h+1 is emitted before B
of chunk h) so ACT hiccups don't stall the DVE chain; tile sizes graduate
small->large->small to shorten pipeline fill/drain. All ACT ops are chained
with scheduler-only deps to pin the table order.

Engine notes learned from profiling:
 - GpSimd tensor ops lock the SBUF port shared with DVE and stall concurrent
   DVE ops for their full duration -> GpSimd only does SWDGE DMA descgen.
 - A same-operand multiply (th*th) runs at 1x; a copy + distinct-operand
   multiply (4x + 2x) is faster, and ACT Square is used where ACT has slack.
 - bf16 keeps tensor_tensor at 2x and tensor_scalar at 4x; inputs are cast
   f32->bf16 in-flight by the SWDGE DMAs (validated: 3.5e-4 rel error).
"""

import numpy as np

import concourse.bacc as bacc
import concourse.mybir as mybir
import concourse.tile as tile
from concourse.tile_rust import add_dep_helper
from concourse.bass_utils import run_bass_kernel_spmd

B, T, K = 16, 131072, 16
N = B * T                 # 2097152 rows total
NCORES = 8
NLOC = N // NCORES        # 262144 rows per core
P = 128                   # SBUF partitions

F32 = mybir.dt.float32
BF16 = mybir.dt.bfloat16
AF = mybir.ActivationFunctionType
OP = mybir.AluOpType


def build_kernel(nloc=NLOC, chunks=None):
    """Build the per-core Bass module.

    chunks: list of tuples of per-tile row counts (rows per partition).
    Each chunk runs phase A (ln/exp side) then phase B (tanh side); sizes
    graduate small->large->small to shorten pipeline fill and drain.
    """
    p = P
    r = nloc // p             # rows per partition
    if chunks is None:
        chunks = [(32, 64), (96, 192), (192, 192), (192, 192), (192, 192),
                  (192, 192), (128,)]
    assert sum(sum(ch) for ch in chunks) == r and nloc % p == 0
    cmax = max(max(ch) for ch in chunks)
    # th^2 on ACT (Square is in every table set) for the larger tiles keeps
    # DVE and ACT balanced; smaller tiles square on DVE via copy+mul.
    act_square_budget = 6

    nc = bacc.Bacc("TRN2", target_bir_lowering=False, debug=False)
    w_d = nc.dram_tensor("w", [nloc, K], F32, kind="ExternalInput")
    loc_d = nc.dram_tensor("loc", [nloc, K], F32, kind="ExternalInput")
    scale_d = nc.dram_tensor("scale", [nloc, K], F32, kind="ExternalInput")
    t_d = nc.dram_tensor("t", [nloc], F32, kind="ExternalInput")
    out_d = nc.dram_tensor("out", [p, 2], F32, kind="ExternalOutput")

    wv = w_d.ap().rearrange("(p r) k -> p r k", p=p)
    lv = loc_d.ap().rearrange("(p r) k -> p r k", p=p)
    sv = scale_d.ap().rearrange("(p r) k -> p r k", p=p)
    tv = t_d.ap().rearrange("(p r) -> p r", p=p)

    acts = []  # every ACT instruction, in required execution order

    def act(*args, **kwargs):
        ins = nc.scalar.activation(*args, **kwargs)
        acts.append(ins)
        return ins

    with tile.TileContext(nc) as tc:
        with (
            tc.tile_pool(name="persist", bufs=1) as pp,
            tc.tile_pool(name="psc", bufs=3) as psc,
            tc.tile_pool(name="pwld", bufs=4) as pwld,
            tc.tile_pool(name="plc", bufs=4) as plc,
            tc.tile_pool(name="prp", bufs=3) as prp,
            tc.tile_pool(name="pv", bufs=6) as pv,
            tc.tile_pool(name="ppw", bufs=6) as ppw,
            tc.tile_pool(name="pc1", bufs=2) as pc1,
            tc.tile_pool(name="pt", bufs=2) as pt,
            nc.allow_low_precision("bf16 partial sums validated: 3.5e-4 rel"),
        ):
            t_all = pp.tile([p, r], F32)          # targets
            stash_s = pp.tile([p, r], F32)        # per-row numerator sums
            stash_w = pp.tile([p, r], F32)        # per-row denominator sums
            out_sb = pp.tile([p, 2], F32)

            self_sq = [0]

            def tree16(src, dst_slice, c):
                """Sum src [p, c, 16] bf16 over last axis -> dst_slice [p, c] f32."""
                t1 = pt.tile([p, cmax, 8], BF16, tag="t1", name="t1")[:, :c, :]
                nc.vector.tensor_add(out=t1, in0=src[:, :, 0:8], in1=src[:, :, 8:16])
                t2 = pt.tile([p, cmax, 4], BF16, tag="t2", name="t2")[:, :c, :]
                nc.vector.tensor_add(out=t2, in0=t1[:, :, 0:4], in1=t1[:, :, 4:8])
                t3 = pt.tile([p, cmax, 2], BF16, tag="t3", name="t3")[:, :c, :]
                nc.vector.tensor_add(out=t3, in0=t2[:, :, 0:2], in1=t2[:, :, 2:4])
                nc.vector.tensor_add(out=dst_slice, in0=t3[:, :, 0], in1=t3[:, :, 1])


            off = 0
            starts = []
            for ch in chunks:
                starts.append(off)
                off += sum(ch)

            def emit_A(ci, ch):
                # ---- phase A of chunk: Ln x2, Exp x4, diff/v/pw/den-sum ----
                tinfo = []
                o = starts[ci]
                csl = slice(o, o + sum(ch))
                nc.gpsimd.dma_start(out=t_all[:, csl], in_=tv[:, csl])
                for c in ch:
                    sl = slice(o, o + c)
                    o += c
                    sc_t = psc.tile([p, cmax, K], BF16, tag="sc", name="sc")[:, :c, :]
                    w_t = pwld.tile([p, cmax, K], BF16, tag="w", name="wt")[:, :c, :]
                    loc_t = plc.tile([p, cmax, K], BF16, tag="loc", name="loct")[:, :c, :]
                    # SWDGE DMAs cast f32->bf16 in flight
                    nc.gpsimd.dma_start(out=sc_t, in_=sv[:, sl, :])
                    nc.gpsimd.dma_start(out=w_t, in_=wv[:, sl, :])
                    nc.gpsimd.dma_start(out=loc_t, in_=lv[:, sl, :])
                    tinfo.append((sl, c, sc_t, w_t, loc_t))

                # all Lns first, then all Exps: walrus maps ln and exp to
                # different table sets, so batching halves the table loads
                for sl, c, sc_t, w_t, loc_t in tinfo:
                    act(out=sc_t, in_=sc_t, func=AF.Ln)          # u, in place
                rps = []
                for sl, c, sc_t, w_t, loc_t in tinfo:
                    rp_t = prp.tile([p, cmax, K], BF16, tag="rp", name="rpt")[:, :c, :]
                    act(out=rp_t, in_=sc_t, func=AF.Exp, scale=-1.0)   # 1/s
                    act(out=w_t, in_=w_t, func=AF.Exp)           # e^w, in place
                    rps.append(rp_t)

                binfo = []
                for (sl, c, sc_t, w_t, loc_t), rp_t in zip(tinfo, rps):
                    # diff = t - loc (broadcast over K), in place over loc.
                    # 1x mode (broadcast AP), but on DVE: GpSimd tensor ops
                    # lock the shared SBUF port and stall concurrent DVE ops
                    # for their full duration, which costs more than this.
                    tb = t_all[:, sl].unsqueeze(2).broadcast_to([p, c, K])
                    nc.vector.tensor_sub(out=loc_t, in0=tb, in1=loc_t)

                    v_t = pv.tile([p, cmax, K], BF16, tag="v", name="vt")[:, :c, :]
                    nc.vector.tensor_mul(out=v_t, in0=loc_t, in1=rp_t)
                    pw_t = ppw.tile([p, cmax, K], BF16, tag="pw", name="pwt")[:, :c, :]
                    nc.vector.tensor_mul(out=pw_t, in0=rp_t, in1=w_t)
                    tree16(w_t, stash_w[:, sl], c)               # sum e^w
                    binfo.append((sl, c, v_t, pw_t))
                return binfo

            def emit_B(binfo):
                # ---- phase B of chunk: tanh + term + num-sum ----
                for sl, c, v_t, pw_t in binfo:
                    act(out=v_t, in_=v_t, func=AF.Tanh, scale=0.5)     # th
                for sl, c, v_t, pw_t in binfo:
                    if c == cmax and self_sq[0] < act_square_budget:
                        # ACT square: Square is in every table set, no reload
                        self_sq[0] += 1
                        c1 = pc1.tile([p, cmax, K], BF16, tag="c1", name="c1t")[:, :c, :]
                        act(out=c1, in_=v_t, func=AF.Square)           # th^2
                    else:
                        c1 = pc1.tile([p, cmax, K], BF16, tag="c1", name="c1t")[:, :c, :]
                        # copy so the square reads two distinct operands (the
                        # same-operand form th*th drops DVE to 1x mode)
                        nc.vector.tensor_copy(out=c1, in_=v_t)         # th
                        nc.vector.tensor_mul(out=c1, in0=c1, in1=v_t)  # th^2
                    nc.vector.tensor_scalar(
                        out=c1, in0=c1, scalar1=-0.25, scalar2=0.25,
                        op0=OP.mult, op1=OP.add,
                    )                                            # (1-th^2)/4
                    nc.vector.tensor_mul(out=v_t, in0=c1, in1=pw_t)    # term
                    tree16(v_t, stash_s[:, sl], c)

            # Software pipeline: emit A of chunk h+1 before B of chunk h so
            # ACT hiccups (table loads, DMA waits) don't stall the DVE chain.
            pending = None
            for ci, ch in enumerate(chunks):
                binfo = emit_A(ci, ch)
                if pending is not None:
                    emit_B(pending)
                pending = binfo
            emit_B(pending)

            # ---- phase C: per-row logs + per-partition accumulation ----
            act(out=stash_s, in_=stash_s, func=AF.Ln, accum_out=out_sb[:, 0:1])
            act(out=stash_w, in_=stash_w, func=AF.Ln, accum_out=out_sb[:, 1:2])
            nc.gpsimd.dma_start(out=out_d.ap(), in_=out_sb)

            # Pin ACT execution order (same engine -> scheduler-only edges)
            for prev, nxt in zip(acts, acts[1:]):
                add_dep_helper(nxt.ins, prev.ins, False, "act-table-order")

    nc.compile()
    return nc


def _combine(outs, n_rows):
    total = 0.0
    for o in outs:
        total += float(o[:, 0].sum(dtype=np.float64))
        total -= float(o[:, 1].sum(dtype=np.float64))
    return np.float32(total / n_rows)


def make_in_maps(weight, loc, scale, targets):
    w = np.ascontiguousarray(weight.reshape(N, K), dtype=np.float32)
    l = np.ascontiguousarray(loc.reshape(N, K), dtype=np.float32)
    s = np.ascontiguousarray(scale.reshape(N, K), dtype=np.float32)
    t = np.ascontiguousarray(targets.reshape(N), dtype=np.float32)
    in_maps = []
    for ci in range(NCORES):
        rs = slice(ci * NLOC, (ci + 1) * NLOC)
        in_maps.append({
            "w": np.ascontiguousarray(w[rs]),
            "loc": np.ascontiguousarray(l[rs]),
            "scale": np.ascontiguousarray(s[rs]),
            "t": np.ascontiguousarray(t[rs]),
        })
    return in_maps


def run(in_maps, **kwargs):
    nc = build_kernel()
    return run_bass_kernel_spmd(nc, in_maps, core_ids=list(range(NCORES)), **kwargs)


def kernel(weight, loc, scale, targets):
    in_maps = make_in_maps(weight, loc, scale, targets)
    last = None
    for _ in range(3):  # rare transient NRT device errors: retry
        try:
            res = run(in_maps)
            return _combine([r["out"] for r in res.results], N)
        except Exception as e:  # noqa: BLE001
            last = e
    raise last


if __name__ == "__main__":
    nc = build_kernel()
    print("kernel built OK")

